# revision 1
# baseline (speedup 1.0000x reference)
"""Trainium2 Bass kernel for the DGGGL graph-conv GRU cell (gnn_message_passing).

Strategy: data-parallel over batch B=8 across the 8 NeuronCores (one batch
element per core, no collectives).  The heavy work is 8 products A @ X with
A [2048, 2048] per (support, cheb-step, agcn): we host-pretranspose A so the
device computes (A @ X)^T = X^T A^T with the small [128, 66] feature tile as
the PE-stationary operand and A^T streamed at F=512 (fp8 DoubleRow).  Both
supports' A^T live resident in SBUF as e4m3 (32KB/partition each), loaded
from HBM exactly once.
Everything accumulates in fp32 PSUM; elementwise/dense chains are fp32/bf16
mixed (validated ~6e-3 max-rel vs the fp32 reference).

Feature order on device is [s1 (0:32), s2 (32:64), xt (64:66)] so that
partition-dim slices land on 32-aligned boundaries (HW requirement); the
host permutes weight rows to match.
"""

import contextlib

import numpy as np
import ml_dtypes

import concourse.bass as bass
import concourse.mybir as mybir
import concourse.tile as tile
from concourse import bacc
from concourse.bass_utils import run_bass_kernel_spmd
from concourse.masks import make_identity

BF16 = mybir.dt.bfloat16
F8 = mybir.dt.float8e4
F32 = mybir.dt.float32
AF = mybir.ActivationFunctionType
DR = mybir.MatmulPerfMode.DoubleRow

P = 128          # partitions
N = 2048         # nodes
NK = N // P      # 16 k-chunks
C = 66           # feature dim into the AGCNs (2 + 32 + 32)
CP = 80          # padded nat-chunk stride (DoubleRow needs step % 16 == 0)
CIN = 74         # MLP gate input dim
G = 96           # 3 * DO
DO = 32
FB = 512         # matmul moving free-dim chunk (one PSUM bank of fp32)
NF = N // FB     # 4
NCORES = 8

# fp8 scaling (exact powers of two): A is shipped as A*SA in e4m3 (softmax
# values <=1 so max 64 < 240); T1/U1 natural copies carry 2*ST so the next
# product's PSUM holds (SA*2*ST) * A@T1 and T2 = psum/(SA*ST) - X.
SA = 64.0
ST = 16.0

_NC_CACHE = None


def _build_bass(reps=1):
    nc = bacc.Bacc("TRN2", target_bir_lowering=False, debug=False)

    # ---- DRAM I/O -------------------------------------------------------
    aT_d = nc.dram_tensor("aT", [2, N, N], F8, kind="ExternalInput")
    xnat_d = nc.dram_tensor("xnat", [N, C], F8, kind="ExternalInput")
    gin_d = nc.dram_tensor("gin", [CIN, N], BF16, kind="ExternalInput")
    sdiff_d = nc.dram_tensor("sdiff", [DO, N], BF16, kind="ExternalInput")
    mlpw_d = nc.dram_tensor("mlp_w", [CIN, DO], BF16, kind="ExternalInput")
    gwc_d = nc.dram_tensor("gwc", [5, C, G], BF16, kind="ExternalInput")
    uwc_d = nc.dram_tensor("uwc", [5, C, DO], BF16, kind="ExternalInput")
    hopw_d = nc.dram_tensor("hop_w", [DO, DO], F32, kind="ExternalInput")
    mlpb_d = nc.dram_tensor("mlp_b", [DO, 1], F32, kind="ExternalInput")
    gateb_d = nc.dram_tensor("gate_b", [G, 1], F32, kind="ExternalInput")
    updb_d = nc.dram_tensor("upd_b", [DO, 1], F32, kind="ExternalInput")
    hopb_d = nc.dram_tensor("hop_b", [DO, 1], F32, kind="ExternalInput")
    hT_d = nc.dram_tensor("h_T", [DO, N], F32, kind="ExternalOutput")
    tnT_d = nc.dram_tensor("tn_T", [DO, N], F32, kind="ExternalOutput")

    with tile.TileContext(nc) as tc:
        with (
            tc.tile_pool(name="const", bufs=1) as const,
            tc.tile_pool(name="abuf", bufs=1) as abuf,
            tc.tile_pool(name="natp", bufs=1) as natp,
            tc.tile_pool(name="tnatp", bufs=2) as tnatp,
            tc.tile_pool(name="termp", bufs=5) as termp,
            tc.tile_pool(name="f32p", bufs=1) as f32p,
            tc.tile_pool(name="pp", bufs=4, space="PSUM") as pp,
            tc.tile_pool(name="tp", bufs=2, space="PSUM") as tp,
            tc.tile_pool(name="dp", bufs=2, space="PSUM") as dp,
        ):
            # ---- constants / weights (loaded once, outside the loop) ---
            ident = const.tile([P, P], BF16)
            make_identity(nc, ident)
            mlpw = const.tile([CIN, DO], BF16)
            nc.sync.dma_start(out=mlpw, in_=mlpw_d[:, :])
            gwc = const.tile([C, 5 * G], BF16)
            uwc = const.tile([C, 5 * DO], BF16)
            for t in range(5):
                nc.sync.dma_start(out=gwc[:, t * G:(t + 1) * G], in_=gwc_d[t])
                nc.sync.dma_start(out=uwc[:, t * DO:(t + 1) * DO], in_=uwc_d[t])
            hopw = const.tile([DO, DO], F32)
            nc.sync.dma_start(out=hopw, in_=hopw_d[:, :])
            mlpb = const.tile([DO, 1], F32)
            gateb = const.tile([G, 1], F32)
            updb = const.tile([DO, 1], F32)
            hopb = const.tile([DO, 1], F32)
            nc.sync.dma_start(out=mlpb, in_=mlpb_d[:, :])
            nc.sync.dma_start(out=gateb, in_=gateb_d[:, :])
            nc.sync.dma_start(out=updb, in_=updb_d[:, :])
            nc.sync.dma_start(out=hopb, in_=hopb_d[:, :])

            loop = tc.For_i(0, reps) if reps > 1 else contextlib.nullcontext()
            with loop:
                _emit_body(
                    nc, tc, aT_d, xnat_d, gin_d, sdiff_d, hT_d, tnT_d,
                    ident, mlpw, gwc, uwc, hopw, mlpb, gateb, updb, hopb,
                    const, abuf, natp, tnatp, termp, f32p, pp, tp, dp,
                )

    nc.compile()
    return nc


def _emit_body(nc, tc, aT_d, xnat_d, gin_d, sdiff_d, hT_d, tnT_d,
               ident, mlpw, gwc, uwc, hopw, mlpb, gateb, updb, hopb,
               const, abuf, natp, tnatp, termp, f32p, pp, tp, dp):
    # ---- activations in -------------------------------------------------
    gin = const.tile([CIN, N], BF16)
    nc.sync.dma_start(out=gin, in_=gin_d[:, :])
    xT = gin[0:C, :]          # X^T is the first 66 rows of gate_in^T
    # DVE tensor-tensor ops need all operands on the SAME start partition,
    # so keep partition-0-aligned copies of s1-s2, s2, and (later) r.
    sdiff = const.tile([DO, N], BF16)
    nc.sync.dma_start(out=sdiff, in_=sdiff_d[:, :])
    s2a = const.tile([DO, N], BF16)
    nc.sync.dma_start(out=s2a, in_=gin_d[DO:2 * DO, :])
    xnat = natp.tile([P, NK, CP], F8)
    for k in range(NK):
        nc.sync.dma_start(
            out=xnat[:, k, 0:C], in_=xnat_d[k * P:(k + 1) * P, :]
        )

    # ---- resident adjacency (transposed, fp8, pre-scaled by SA) ---------
    a_res = []
    for s in range(2):
        at = abuf.tile([P, NK, N], F8, tag=f"a{s}", name=f"a{s}")
        for k in range(NK):
            nc.sync.dma_start(out=at[:, k, :], in_=aT_d[s, k * P:(k + 1) * P, :])
        a_res.append(at)

    # ---- MLP mixing gate + state ----------------------------------------
    mr = f32p.tile([DO, N], F32, tag="mrhc", name="mr")
    for f in range(NF):
        fs = slice(f * FB, (f + 1) * FB)
        ps = dp.tile([G, FB], F32, tag="dp", name="ps_mlp")
        nc.tensor.matmul(ps[0:DO, :], mlpw, gin[:, fs], start=True, stop=True)
        nc.scalar.activation(mr[:, fs], ps[0:DO, :], AF.Sigmoid, bias=mlpb)
    state = f32p.tile([DO, N], F32, tag="state_tn", name="state")
    nc.vector.tensor_mul(state, mr, sdiff)
    nc.vector.tensor_add(state, state, s2a)

    # ---- helpers --------------------------------------------------------
    def product(nat_tile, s, evict, k_outer=False):
        """psum[c, f] = sum_k nat_chunk_k^T @ A_s^T[k, f]; evict(f, psum).
        fp8 DoubleRow: each matmul contracts a PAIR of 128-row chunks.
        F-outer evicts chunk-by-chunk (pipelines downstream transposes);
        K-outer holds 4 PSUM banks and loads each pair-weight once."""
        if k_outer:
            pss = [pp.tile([C, FB], F32, tag="pp", name=f"ps_prod{f}")
                   for f in range(NF)]
            for kp in range(NK // 2):
                for f in range(NF):
                    nc.tensor.matmul(
                        pss[f],
                        nat_tile[:, 2 * kp:2 * kp + 2, 0:C],
                        a_res[s][:, 2 * kp:2 * kp + 2, f * FB:(f + 1) * FB],
                        start=(kp == 0),
                        stop=(kp == NK // 2 - 1),
                        perf_mode=DR,
                    )
            for f in range(NF):
                evict(f, pss[f])
            return
        for f in range(NF):
            fs = slice(f * FB, (f + 1) * FB)
            ps = pp.tile([C, FB], F32, tag="pp", name="ps_prod")
            for kp in range(NK // 2):
                nc.tensor.matmul(
                    ps,
                    nat_tile[:, 2 * kp:2 * kp + 2, 0:C],
                    a_res[s][:, 2 * kp:2 * kp + 2, fs],
                    start=(kp == 0),
                    stop=(kp == NK // 2 - 1),
                    perf_mode=DR,
                )
            evict(f, ps)

    def transpose_to_nat(src_T, dst_nat, scale, ks):
        """bf16 [C, 128] cols of src_T -> fp8 natural chunks (times scale).
        Copies alternate ACT/DVE so neither engine gates the PE pipeline."""
        for k in ks:
            tps = tp.tile([P, C], BF16, tag="tp", name="tps")
            nc.tensor.transpose(tps, src_T[:, k * P:(k + 1) * P], ident[0:C, 0:C])
            if k % 2 == 0:
                nc.vector.tensor_scalar_mul(dst_nat[:, k, 0:C], tps, scale)
            else:
                nc.scalar.activation(dst_nat[:, k, 0:C], tps, AF.Copy, scale=scale)

    def agcn(src_nat, src_T, wc, out_cb):
        """Both supports' T1/T2 products, then the dense matmul over the 5
        terms [src, T1a, T2a, T1b, T2b]; out_cb(f, psum) evicts."""
        terms = []
        for s in range(2):
            t1 = termp.tile([C, N], BF16, tag="term", name=f"t1_{s}")
            t1n2 = tnatp.tile([P, NK, CP], F8, tag="tnat", name=f"t1n2_{s}")

            def evict_t1(f, ps, t1=t1, t1n2=t1n2):
                # psum holds SA * A@src; un-scale the bf16 term copy, then
                # immediately transpose this F-chunk's 4 k-blocks so T2's
                # stationary fp8 chunks (2*ST * T1) become ready pipelined.
                nc.scalar.activation(
                    t1[:, f * FB:(f + 1) * FB], ps, AF.Copy, scale=1.0 / SA
                )
                kpf = FB // P
                transpose_to_nat(t1, t1n2, 2.0 * ST, range(f * kpf, (f + 1) * kpf))

            product(src_nat, s, evict_t1)
            t2 = termp.tile([C, N], BF16, tag="term", name=f"t2_{s}")
            product(
                t1n2, s,
                # psum = (SA*2*ST) A@T1; T2 = psum/(SA*ST) - src
                lambda f, ps, t2=t2: nc.vector.scalar_tensor_tensor(
                    t2[:, f * FB:(f + 1) * FB], ps, 1.0 / (SA * ST),
                    src_T[:, f * FB:(f + 1) * FB],
                    op0=mybir.AluOpType.mult, op1=mybir.AluOpType.subtract,
                ),
            )
            terms.extend([t1, t2])
        rhs_all = [src_T, terms[0], terms[1], terms[2], terms[3]]
        od = wc.shape[1] // 5
        for f in range(NF):
            fs = slice(f * FB, (f + 1) * FB)
            ps = dp.tile([G, FB], F32, tag="dp", name="ps_dense")
            for t, rt in enumerate(rhs_all):
                nc.tensor.matmul(
                    ps[0:od, :], wc[:, t * od:(t + 1) * od], rt[:, fs],
                    start=(t == 0), stop=(t == 4),
                )
            out_cb(f, ps[0:od, :])

    # ---- AGCN 1: gates --------------------------------------------------
    zz = const.tile([G, N], BF16)
    agcn(
        xnat, xT, gwc,
        lambda f, ps: nc.scalar.activation(
            zz[:, f * FB:(f + 1) * FB], ps, AF.Sigmoid, bias=gateb
        ),
    )

    # ---- candidate (chunked per F so transposes pipeline behind gate) ---
    candT = termp.tile([C, N], BF16, tag="term", name="candT")
    cnat = natp.tile([P, NK, CP], F8)
    for f in range(NF):
        fs = slice(f * FB, (f + 1) * FB)
        nc.vector.tensor_mul(candT[0:DO, fs], zz[0:DO, fs], gin[0:DO, fs])
        nc.vector.tensor_mul(candT[DO:2 * DO, fs], zz[DO:2 * DO, fs], gin[DO:2 * DO, fs])
        nc.vector.tensor_copy(candT[2 * DO:C, fs], gin[2 * DO:C, fs])
        kpf = FB // P
        transpose_to_nat(candT, cnat, 1.0, range(f * kpf, (f + 1) * kpf))

    # ---- AGCN 2: candidate hc ------------------------------------------
    hc = f32p.tile([DO, N], F32, tag="mrhc", name="hc")
    agcn(
        cnat, candT, uwc,
        lambda f, ps: nc.scalar.activation(
            hc[:, f * FB:(f + 1) * FB], ps, AF.Tanh, bias=updb
        ),
    )

    # ---- h = r*state + (1-r)*hc = hc + r*(state-hc); then hop matmul ----
    # all chunked per F so the tail pipelines with the upd matmuls.
    # r lives at partitions 64:96 of zz; DMA-shift it to 0:32.
    rT = const.tile([DO, N], BF16)
    nc.sync.dma_start(out=rT, in_=zz[2 * DO:3 * DO, :])
    h = f32p.tile([DO, N], F32, tag="h", name="h")
    tn = f32p.tile([DO, N], F32, tag="state_tn", name="tn")
    for f in range(NF):
        fs = slice(f * FB, (f + 1) * FB)
        nc.vector.tensor_sub(h[:, fs], state[:, fs], hc[:, fs])
        nc.vector.tensor_mul(h[:, fs], h[:, fs], rT[:, fs])
        nc.vector.tensor_add(h[:, fs], h[:, fs], hc[:, fs])
        nc.sync.dma_start(out=hT_d[:, fs], in_=h[:, fs])
        ps = dp.tile([G, FB], F32, tag="dp", name="ps_hop")
        nc.tensor.matmul(ps[0:DO, :], hopw, h[:, fs], start=True, stop=True)
        nc.scalar.activation(tn[:, fs], ps[0:DO, :], AF.Identity, bias=hopb)
        nc.sync.dma_start(out=tnT_d[:, fs], in_=tn[:, fs])


def _get_nc():
    global _NC_CACHE
    if _NC_CACHE is None:
        _NC_CACHE = _build_bass()
    return _NC_CACHE


def _host_prep(inputs):
    bf = ml_dtypes.bfloat16
    xt = np.asarray(inputs["xt"], np.float32)
    s1 = np.asarray(inputs["state1"], np.float32)
    s2 = np.asarray(inputs["state2"], np.float32)
    ge = np.asarray(inputs["gatembedding"], np.float32)
    sup = np.asarray(inputs["supports"], np.float32)
    gw = np.asarray(inputs["gate_w"], np.float32)
    uw = np.asarray(inputs["upd_w"], np.float32)

    # feature order on device is [s1, s2, xt]; permute weight rows to match.
    perm66 = np.concatenate([np.arange(2, 34), np.arange(34, 66), np.arange(0, 2)])
    perm74 = np.concatenate([perm66, np.arange(66, 74)])

    # combined X-term weights (T0 appears once per support at rows 0:66, 198:264)
    gwc = np.stack([gw[0:C] + gw[3 * C:4 * C], gw[C:2 * C], gw[2 * C:3 * C],
                    gw[4 * C:5 * C], gw[5 * C:6 * C]])[:, perm66].astype(bf)
    uwc = np.stack([uw[0:C] + uw[3 * C:4 * C], uw[C:2 * C], uw[2 * C:3 * C],
                    uw[4 * C:5 * C], uw[5 * C:6 * C]])[:, perm66].astype(bf)
    shared = {
        "mlp_w": np.asarray(inputs["mlp_w"], np.float32)[perm74].astype(bf),
        "gwc": gwc,
        "uwc": uwc,
        "hop_w": np.asarray(inputs["hop_w"], np.float32),
        "mlp_b": np.asarray(inputs["mlp_b"], np.float32).reshape(DO, 1),
        "gate_b": np.asarray(inputs["gate_b"], np.float32).reshape(G, 1),
        "upd_b": np.asarray(inputs["upd_b"], np.float32).reshape(DO, 1),
        "hop_b": np.asarray(inputs["hop_b"], np.float32).reshape(DO, 1),
    }
    f8 = ml_dtypes.float8_e4m3
    in_maps = []
    for b in range(NCORES):
        x_cat = np.concatenate([s1[b], s2[b], xt[b]], axis=-1)        # [N, 66]
        gin_cat = np.concatenate([s1[b], s2[b], xt[b], ge[b]], axis=-1)
        in_maps.append({
            "aT": (np.ascontiguousarray(sup[:, b].transpose(0, 2, 1)) * SA).astype(f8),
            "xnat": x_cat.astype(f8),
            "gin": np.ascontiguousarray(gin_cat.T).astype(bf),
            "sdiff": np.ascontiguousarray((s1[b] - s2[b]).T).astype(bf),
            **shared,
        })
    return in_maps


def _run(inputs, **kw):
    # Under axon, BASS_TRACE=1 makes run_bass_kernel_spmd import the NTFF
    # hook module, which trimmed containers lack; fail soft to no-trace.
    try:
        from concourse._compat import axon_active
        if axon_active():
            import antenv.axon_hooks  # noqa: F401
    except ImportError:
        import os
        os.environ.setdefault("BASS_NEVER_TRACE", "1")
    nc = _get_nc()
    in_maps = _host_prep(inputs)
    res = run_bass_kernel_spmd(nc, in_maps, core_ids=list(range(NCORES)), **kw)
    h = np.stack([np.asarray(r["h_T"], np.float32).T for r in res.results])
    tn = np.stack([np.asarray(r["tn_T"], np.float32).T for r in res.results])
    return (h, tn), res


def kernel(**inputs):
    return _run(inputs)[0]



# revision 3
# speedup vs baseline: 1.2929x; 1.2929x over previous
"""Trainium2 Bass kernel for the DGGGL graph-conv GRU cell (gnn_message_passing).

Strategy: data-parallel over batch B=8 across the 8 NeuronCores (one batch
element per core, no collectives).  The heavy work is 8 products A @ X with
A [2048, 2048] per (support, cheb-step, agcn): we host-pretranspose A so the
device computes (A @ X)^T = X^T A^T with the small [128, 66] feature tile as
the PE-stationary operand and A^T streamed at F=512 (fp8 DoubleRow).  Both
supports' A^T live resident in SBUF as e4m3 (32KB/partition each), loaded
from HBM exactly once.

v2 scheduling: DMA priority order (small activations, then A0, then A1) with
k-outer products whose kp-pair consumption order matches the row-block DMA
arrival order, so the first product starts ~1.5us in and is paced by the A0
stream instead of waiting for it.  Per AGCN the PE order is T1a,[T1aT],T2a,
T1b,[T1bT],T2b for the DMA-paced first AGCN and T1a,T1b,T1aT,T2a,T1bT,T2b
for the resident second one, hiding each transpose/evict chain behind the
next product.  Transposes land 4-at-a-time in one PSUM bank at the 80-column
nat stride so a single ACT/DVE copy converts each group to fp8.  Elementwise
chains (state mix, candidate gating, final GRU blend) run on the Pool engine
to keep ACT/DVE free for PSUM evictions.

Feature order on device is [s1 (0:32), s2 (32:64), xt (64:66)] so that
partition-dim slices land on 32-aligned boundaries (HW requirement); the
host permutes weight rows to match.
"""

import contextlib

import numpy as np
import ml_dtypes

import concourse.bass as bass
import concourse.mybir as mybir
import concourse.tile as tile
from concourse import bacc
from concourse.bass_utils import run_bass_kernel_spmd
from concourse.masks import make_identity

BF16 = mybir.dt.bfloat16
F8 = mybir.dt.float8e4
F32 = mybir.dt.float32
AF = mybir.ActivationFunctionType
DR = mybir.MatmulPerfMode.DoubleRow

P = 128          # partitions
N = 2048         # nodes
NK = N // P      # 16 k-chunks
C = 66           # feature dim into the AGCNs (2 + 32 + 32)
CP = 80          # padded nat-chunk stride (DoubleRow needs step % 16 == 0)
CIN = 74         # MLP gate input dim
G = 96           # 3 * DO
DO = 32
FB = 512         # matmul moving free-dim chunk (one PSUM bank of fp32)
NF = N // FB     # 4
KPF = FB // P    # 4 k-chunks per f-chunk
NCORES = 8

# fp8 scaling (exact powers of two): A is shipped as A*SA in e4m3 (softmax
# values <=1 so max 64 < 240); T1/U1 natural copies carry 2*ST so the next
# product's PSUM holds (SA*2*ST) * A@T1 and T2 = psum/(SA*ST) - X.
SA = 64.0
ST = 16.0

_NC_CACHE = None


def _build_bass(reps=1):
    nc = bacc.Bacc("TRN2", target_bir_lowering=False, debug=False)

    # ---- DRAM I/O -------------------------------------------------------
    aT_d = nc.dram_tensor("aT", [2, N, N], F8, kind="ExternalInput")
    xnat_d = nc.dram_tensor("xnat", [N, C], F8, kind="ExternalInput")
    gin_d = nc.dram_tensor("gin", [CIN, N], BF16, kind="ExternalInput")
    sdiff_d = nc.dram_tensor("sdiff", [DO, N], BF16, kind="ExternalInput")
    mlpw_d = nc.dram_tensor("mlp_w", [CIN, DO], BF16, kind="ExternalInput")
    gwc_d = nc.dram_tensor("gwc", [5, C, G], BF16, kind="ExternalInput")
    uwc_d = nc.dram_tensor("uwc", [5, C, DO], BF16, kind="ExternalInput")
    hopw_d = nc.dram_tensor("hop_w", [DO, DO], F32, kind="ExternalInput")
    mlpb_d = nc.dram_tensor("mlp_b", [DO, 1], F32, kind="ExternalInput")
    gateb_d = nc.dram_tensor("gate_b", [G, 1], F32, kind="ExternalInput")
    updb_d = nc.dram_tensor("upd_b", [DO, 1], F32, kind="ExternalInput")
    hopb_d = nc.dram_tensor("hop_b", [DO, 1], F32, kind="ExternalInput")
    hT_d = nc.dram_tensor("h_T", [DO, N], F32, kind="ExternalOutput")
    tnT_d = nc.dram_tensor("tn_T", [DO, N], F32, kind="ExternalOutput")

    with tile.TileContext(nc) as tc:
        with (
            tc.tile_pool(name="const", bufs=1) as const,
            tc.tile_pool(name="abuf", bufs=1) as abuf,
            tc.tile_pool(name="natp", bufs=1) as natp,
            tc.tile_pool(name="tnatp", bufs=2) as tnatp,
            tc.tile_pool(name="termp", bufs=5) as termp,
            tc.tile_pool(name="f32p", bufs=1) as f32p,
            tc.tile_pool(name="pp", bufs=4, space="PSUM") as pp,
            tc.tile_pool(name="tp", bufs=2, space="PSUM") as tp,
            tc.tile_pool(name="dp", bufs=2, space="PSUM") as dp,
        ):
            # ---- constants / weights (loaded once, outside the loop) ---
            ident = const.tile([P, P], BF16)
            make_identity(nc, ident)
            mlpw = const.tile([CIN, DO], BF16)
            nc.sync.dma_start(out=mlpw, in_=mlpw_d[:, :])
            gwc = const.tile([C, 5 * G], BF16)
            uwc = const.tile([C, 5 * DO], BF16)
            for t in range(5):
                nc.sync.dma_start(out=gwc[:, t * G:(t + 1) * G], in_=gwc_d[t])
                nc.sync.dma_start(out=uwc[:, t * DO:(t + 1) * DO], in_=uwc_d[t])
            hopw = const.tile([DO, DO], F32)
            nc.sync.dma_start(out=hopw, in_=hopw_d[:, :])
            mlpb = const.tile([DO, 1], F32)
            gateb = const.tile([G, 1], F32)
            updb = const.tile([DO, 1], F32)
            hopb = const.tile([DO, 1], F32)
            nc.sync.dma_start(out=mlpb, in_=mlpb_d[:, :])
            nc.sync.dma_start(out=gateb, in_=gateb_d[:, :])
            nc.sync.dma_start(out=updb, in_=updb_d[:, :])
            nc.sync.dma_start(out=hopb, in_=hopb_d[:, :])

            loop = tc.For_i(0, reps) if reps > 1 else contextlib.nullcontext()
            with loop:
                _emit_body(
                    nc, tc, aT_d, xnat_d, gin_d, sdiff_d, hT_d, tnT_d,
                    ident, mlpw, gwc, uwc, hopw, mlpb, gateb, updb, hopb,
                    const, abuf, natp, tnatp, termp, f32p, pp, tp, dp,
                )

    nc.compile()
    return nc


def _emit_body(nc, tc, aT_d, xnat_d, gin_d, sdiff_d, hT_d, tnT_d,
               ident, mlpw, gwc, uwc, hopw, mlpb, gateb, updb, hopb,
               const, abuf, natp, tnatp, termp, f32p, pp, tp, dp):
    # ---- input DMAs in priority order: small activations, then A0, A1 ---
    gin = const.tile([CIN, N], BF16)
    nc.sync.dma_start(out=gin, in_=gin_d[:, :])
    xT = gin[0:C, :]          # X^T is the first 66 rows of gate_in^T
    xnat = natp.tile([P, NK, CP], F8)
    for k in range(NK):
        nc.sync.dma_start(
            out=xnat[:, k, 0:C], in_=xnat_d[k * P:(k + 1) * P, :]
        )
    # DVE/Pool tensor-tensor ops need all operands on the SAME start
    # partition, so keep partition-0-aligned copies of s1-s2 and s2.
    sdiff = const.tile([DO, N], BF16)
    nc.sync.dma_start(out=sdiff, in_=sdiff_d[:, :])
    s2a = const.tile([DO, N], BF16)
    nc.sync.dma_start(out=s2a, in_=gin_d[DO:2 * DO, :])
    # resident adjacency (transposed, fp8, pre-scaled by SA): all of A0
    # first so the first product's kp-order consumption matches arrival.
    a_res = []
    for s in range(2):
        at = abuf.tile([P, NK, N], F8, tag=f"a{s}", name=f"a{s}")
        a_res.append(at)
    for s in range(2):
        for k in range(NK):
            nc.sync.dma_start(
                out=a_res[s][:, k, :], in_=aT_d[s, k * P:(k + 1) * P, :]
            )

    # ---- MLP mixing gate + state (fills the A0 DMA window) --------------
    mr = f32p.tile([DO, N], F32, tag="mrhc", name="mr")
    for f in range(NF):
        fs = slice(f * FB, (f + 1) * FB)
        ps = dp.tile([G, FB], F32, tag="dp", name="ps_mlp")
        nc.tensor.matmul(ps[0:DO, :], mlpw, gin[:, fs], start=True, stop=True)
        nc.scalar.activation(mr[:, fs], ps[0:DO, :], AF.Sigmoid, bias=mlpb)
    state = f32p.tile([DO, N], F32, tag="state_tn", name="state")
    nc.gpsimd.tensor_mul(state, mr, sdiff)
    nc.gpsimd.tensor_add(state, state, s2a)

    # ---- helpers --------------------------------------------------------
    def product(nat_tile, s, evict):
        """psum[c, f] = sum_k nat_chunk_k^T @ A_s^T[k, f]; evict(f, psum).
        fp8 DoubleRow: each matmul contracts a PAIR of 128-row chunks.
        K-outer: holds 4 PSUM banks, loads each pair-weight once, and
        consumes A row-block pairs in DMA arrival order."""
        pss = [pp.tile([C, FB], F32, tag="pp", name=f"ps_prod{f}")
               for f in range(NF)]
        for kp in range(NK // 2):
            for f in range(NF):
                nc.tensor.matmul(
                    pss[f],
                    nat_tile[:, 2 * kp:2 * kp + 2, 0:C],
                    a_res[s][:, 2 * kp:2 * kp + 2, f * FB:(f + 1) * FB],
                    start=(kp == 0),
                    stop=(kp == NK // 2 - 1),
                    perf_mode=DR,
                )
        for f in range(NF):
            evict(f, pss[f])

    def evict_t1(t1, f, ps):
        """psum holds SA * A@src; un-scale into the bf16 term copy,
        alternating ACT/DVE so neither engine serializes the 4 banks."""
        if f % 2 == 0:
            nc.scalar.activation(
                t1[:, f * FB:(f + 1) * FB], ps, AF.Copy, scale=1.0 / SA
            )
        else:
            nc.vector.tensor_scalar_mul(
                t1[:, f * FB:(f + 1) * FB], ps, 1.0 / SA
            )

    def transpose_group(src_T, dst_nat, scale, f):
        """One f-chunk of bf16 [C, N] -> 4 fp8 natural chunks: 4 PE
        transposes into a single PSUM bank at the 80-column nat stride,
        then one ACT/DVE copy (times scale) converts the whole group."""
        tps = tp.tile([P, KPF, CP], BF16, tag="tp", name="tps")
        for j in range(KPF):
            k = KPF * f + j
            nc.tensor.transpose(
                tps[:, j, 0:C], src_T[:, k * P:(k + 1) * P], ident[0:C, 0:C]
            )
        dst = dst_nat[:, KPF * f:KPF * f + KPF, :]
        if f % 2 == 0:
            nc.vector.tensor_scalar_mul(dst, tps, scale)
        else:
            nc.scalar.activation(dst, tps, AF.Copy, scale=scale)

    def evict_t2(t2, src_T, f, ps):
        # psum = (SA*2*ST) A@T1; T2 = psum/(SA*ST) - src
        nc.vector.scalar_tensor_tensor(
            t2[:, f * FB:(f + 1) * FB], ps, 1.0 / (SA * ST),
            src_T[:, f * FB:(f + 1) * FB],
            op0=mybir.AluOpType.mult, op1=mybir.AluOpType.subtract,
        )

    def agcn(src_nat, src_T, wc, out_cb, paced):
        """Both supports' T1/T2 products, then the dense matmul over the 5
        terms [src, T1a, T2a, T1b, T2b]; out_cb(f, psum) evicts.
        paced=True (first AGCN): per-support chains T1s,[T1sT],T2s so T2a
        runs while A1 is still streaming in.  paced=False: T1a,T1b first,
        hiding each transpose/evict chain behind the other support's
        product."""
        t1s = [termp.tile([C, N], BF16, tag="term", name=f"t1_{s}")
               for s in range(2)]
        t2s = [termp.tile([C, N], BF16, tag="term", name=f"t2_{s}")
               for s in range(2)]
        t1n2s = [tnatp.tile([P, NK, CP], F8, tag="tnat", name=f"t1n2_{s}")
                 for s in range(2)]

        def t1_product(s):
            product(src_nat, s, lambda f, ps: evict_t1(t1s[s], f, ps))

        def t1_transposes(s):
            for f in range(NF):
                transpose_group(t1s[s], t1n2s[s], 2.0 * ST, f)

        def t2_product(s):
            product(t1n2s[s], s, lambda f, ps: evict_t2(t2s[s], src_T, f, ps))

        if paced:
            t1_product(0)
            t1_transposes(0)
            t2_product(0)
            t1_product(1)
            t1_transposes(1)
            t2_product(1)
        else:
            t1_product(0)
            t1_product(1)
            t1_transposes(0)
            t2_product(0)
            t1_transposes(1)
            t2_product(1)

        rhs_all = [src_T, t1s[0], t2s[0], t1s[1], t2s[1]]
        od = wc.shape[1] // 5
        for f in range(NF):
            fs = slice(f * FB, (f + 1) * FB)
            ps = dp.tile([G, FB], F32, tag="dp", name="ps_dense")
            for t, rt in enumerate(rhs_all):
                nc.tensor.matmul(
                    ps[0:od, :], wc[:, t * od:(t + 1) * od], rt[:, fs],
                    start=(t == 0), stop=(t == 4),
                )
            out_cb(f, ps[0:od, :])

    # ---- AGCN 1: gates --------------------------------------------------
    zz = const.tile([G, N], BF16)
    candT = termp.tile([C, N], BF16, tag="term", name="candT")
    cnat = natp.tile([P, NK, CP], F8)

    def gate_out(f, ps):
        fs = slice(f * FB, (f + 1) * FB)
        nc.scalar.activation(zz[:, fs], ps, AF.Sigmoid, bias=gateb)
        # candidate = [z1*s1, z2*s2, xt] for this f-chunk, then its 4
        # nat transposes, pipelined behind the next f's dense matmuls.
        nc.gpsimd.tensor_mul(candT[0:DO, fs], zz[0:DO, fs], gin[0:DO, fs])
        nc.gpsimd.tensor_mul(
            candT[DO:2 * DO, fs], zz[DO:2 * DO, fs], gin[DO:2 * DO, fs]
        )
        nc.gpsimd.tensor_copy(candT[2 * DO:C, fs], gin[2 * DO:C, fs])
        transpose_group(candT, cnat, 1.0, f)

    agcn(xnat, xT, gwc, gate_out, paced=True)

    # r lives at partitions 64:96 of zz; DMA-shift it to 0:32.
    rT = const.tile([DO, N], BF16)
    nc.sync.dma_start(out=rT, in_=zz[2 * DO:3 * DO, :])

    # ---- AGCN 2: candidate hc, then h and the hop matmul ----------------
    hc = f32p.tile([DO, N], F32, tag="mrhc", name="hc")
    h = f32p.tile([DO, N], F32, tag="h", name="h")
    tn = f32p.tile([DO, N], F32, tag="tn", name="tn")

    def upd_out(f, ps):
        fs = slice(f * FB, (f + 1) * FB)
        nc.scalar.activation(hc[:, fs], ps, AF.Tanh, bias=updb)
        # h = r*state + (1-r)*hc = hc + r*(state-hc), on Pool
        nc.gpsimd.tensor_sub(h[:, fs], state[:, fs], hc[:, fs])
        nc.gpsimd.tensor_mul(h[:, fs], h[:, fs], rT[:, fs])
        nc.gpsimd.tensor_add(h[:, fs], h[:, fs], hc[:, fs])
        nc.sync.dma_start(out=hT_d[:, fs], in_=h[:, fs])
        psh = dp.tile([G, FB], F32, tag="dp", name="ps_hop")
        nc.tensor.matmul(psh[0:DO, :], hopw, h[:, fs], start=True, stop=True)
        nc.scalar.activation(tn[:, fs], psh[0:DO, :], AF.Identity, bias=hopb)
        nc.sync.dma_start(out=tnT_d[:, fs], in_=tn[:, fs])

    agcn(cnat, candT, uwc, upd_out, paced=False)


def _get_nc():
    global _NC_CACHE
    if _NC_CACHE is None:
        _NC_CACHE = _build_bass()
    return _NC_CACHE


def _host_prep(inputs):
    bf = ml_dtypes.bfloat16
    xt = np.asarray(inputs["xt"], np.float32)
    s1 = np.asarray(inputs["state1"], np.float32)
    s2 = np.asarray(inputs["state2"], np.float32)
    ge = np.asarray(inputs["gatembedding"], np.float32)
    sup = np.asarray(inputs["supports"], np.float32)
    gw = np.asarray(inputs["gate_w"], np.float32)
    uw = np.asarray(inputs["upd_w"], np.float32)

    # feature order on device is [s1, s2, xt]; permute weight rows to match.
    perm66 = np.concatenate([np.arange(2, 34), np.arange(34, 66), np.arange(0, 2)])
    perm74 = np.concatenate([perm66, np.arange(66, 74)])

    # combined X-term weights (T0 appears once per support at rows 0:66, 198:264)
    gwc = np.stack([gw[0:C] + gw[3 * C:4 * C], gw[C:2 * C], gw[2 * C:3 * C],
                    gw[4 * C:5 * C], gw[5 * C:6 * C]])[:, perm66].astype(bf)
    uwc = np.stack([uw[0:C] + uw[3 * C:4 * C], uw[C:2 * C], uw[2 * C:3 * C],
                    uw[4 * C:5 * C], uw[5 * C:6 * C]])[:, perm66].astype(bf)
    shared = {
        "mlp_w": np.asarray(inputs["mlp_w"], np.float32)[perm74].astype(bf),
        "gwc": gwc,
        "uwc": uwc,
        "hop_w": np.asarray(inputs["hop_w"], np.float32),
        "mlp_b": np.asarray(inputs["mlp_b"], np.float32).reshape(DO, 1),
        "gate_b": np.asarray(inputs["gate_b"], np.float32).reshape(G, 1),
        "upd_b": np.asarray(inputs["upd_b"], np.float32).reshape(DO, 1),
        "hop_b": np.asarray(inputs["hop_b"], np.float32).reshape(DO, 1),
    }
    f8 = ml_dtypes.float8_e4m3
    in_maps = []
    for b in range(NCORES):
        x_cat = np.concatenate([s1[b], s2[b], xt[b]], axis=-1)        # [N, 66]
        gin_cat = np.concatenate([s1[b], s2[b], xt[b], ge[b]], axis=-1)
        in_maps.append({
            "aT": (np.ascontiguousarray(sup[:, b].transpose(0, 2, 1)) * SA).astype(f8),
            "xnat": x_cat.astype(f8),
            "gin": np.ascontiguousarray(gin_cat.T).astype(bf),
            "sdiff": np.ascontiguousarray((s1[b] - s2[b]).T).astype(bf),
            **shared,
        })
    return in_maps


def _run(inputs, **kw):
    # Under axon, BASS_TRACE=1 makes run_bass_kernel_spmd import the NTFF
    # hook module, which trimmed containers lack; fail soft to no-trace.
    try:
        from concourse._compat import axon_active
        if axon_active():
            import antenv.axon_hooks  # noqa: F401
    except ImportError:
        import os
        os.environ.setdefault("BASS_NEVER_TRACE", "1")
    nc = _get_nc()
    in_maps = _host_prep(inputs)
    res = run_bass_kernel_spmd(nc, in_maps, core_ids=list(range(NCORES)), **kw)
    h = np.stack([np.asarray(r["h_T"], np.float32).T for r in res.results])
    tn = np.stack([np.asarray(r["tn_T"], np.float32).T for r in res.results])
    return (h, tn), res


def kernel(**inputs):
    return _run(inputs)[0]
